# revision 1
# baseline (speedup 1.0000x reference)
"""Trainium2 Bass kernel for nn_ByteLevelDecoder.

Strategy: data-parallel over the 512 (=B*S) independent byte streams,
64 streams per core on 8 cores. Weights replicated (bf16, gains folded),
KV caches resident in SBUF (bf16; odd layers use partitions 64-127 via
matmul col-offset tile_position so per-partition SBUF stays in budget).
The reference's prefill step (4 seed positions, causal) is mathematically
identical to processing positions 0..3 sequentially one at a time, so the
kernel runs 15 position-iterations total, each = 4 transformer blocks,
with the finished-mask / logits logic on iterations 3..14.
"""

import math
import os

import ml_dtypes
import numpy as np

# ---- problem dims (hardcoded; kernel.py must be self-contained) ----
B, S, H = 2, 256, 1024
BH, NH, HD = 384, 8, 48
P_SEED = 4
S_C = 12
L = 4
V = 258
T = P_SEED + S_C            # 16 (cache capacity)
EOS = 257
SCALE = 1.0 / math.sqrt(HD)
NCORES = 8
NS = (B * S) // NCORES      # 64 streams per core
NIT = P_SEED + S_C - 1      # 15 position iterations (positions 0..14)
F32 = np.float32
BF16 = ml_dtypes.bfloat16


def _pack_inputs(x, Wproj, attn_norm, Wq, Wk, Wv, Wo, ffn_norm, W1, W2, Wlm):
    """Host-side repack: fold norm gains into weight rows, cast to bf16,
    lay out k-tiles with the contraction dim on partitions."""
    x = np.asarray(x, F32).reshape(B * S, H)
    an = np.asarray(attn_norm, F32)      # [L, BH]
    fn = np.asarray(ffn_norm, F32)       # [L, BH]
    Wq, Wk, Wv = (np.asarray(w, F32) for w in (Wq, Wk, Wv))
    Wo, W1, W2 = (np.asarray(w, F32) for w in (Wo, W1, W2))
    Wproj = np.asarray(Wproj, F32)
    Wlm = np.asarray(Wlm, F32)

    # QKV concat, gain folded on input rows: [L, BH, 3*BH] -> [128, L, 3, 3, 384]
    wqkv = np.concatenate([an[:, :, None] * Wq, an[:, :, None] * Wk,
                           an[:, :, None] * Wv], axis=2)      # [L, BH, 1152]
    wqkv = wqkv.reshape(L, 3, 128, 3, BH).transpose(2, 0, 1, 3, 4)  # [128,L,3kt,3j,384]
    # Wo: [L, BH, BH] -> [128, L, 3, 384]
    wo = Wo.reshape(L, 3, 128, BH).transpose(2, 0, 1, 3)
    # W1 (gain folded), stored as lhsT tiles [128k, L, 3kt, 12mt, 128m]
    w1 = (fn[:, :, None] * W1).reshape(L, 3, 128, 12, 128).transpose(2, 0, 1, 3, 4)
    # W2: [L, 4BH, BH] -> [128, L, 12, 384]
    w2 = W2.reshape(L, 12, 128, BH).transpose(2, 0, 1, 3)
    # Wproj: [H, P*BH] -> [128, 8, 1536]
    wproj = Wproj.reshape(8, 128, P_SEED * BH).transpose(1, 0, 2)
    # Wlm: [BH, V] -> [128, 3, 258]
    wlm = Wlm.reshape(3, 128, V).transpose(1, 0, 2)

    ident = np.zeros((128, 64), F32)
    ident[:64] = np.eye(64, dtype=F32)
    ident[64:] = np.eye(64, dtype=F32)

    shared = {
        "wqkv": np.ascontiguousarray(wqkv).astype(BF16),
        "wo": np.ascontiguousarray(wo).astype(BF16),
        "w1": np.ascontiguousarray(w1).astype(BF16),
        "w2": np.ascontiguousarray(w2).astype(BF16),
        "wproj": np.ascontiguousarray(wproj).astype(BF16),
        "wlm": np.ascontiguousarray(wlm).astype(BF16),
        "identf": ident,
        "identb": ident.astype(BF16),
    }
    xs = [np.ascontiguousarray(x[c * NS:(c + 1) * NS]) for c in range(NCORES)]
    return shared, xs


def _build(nc, tc, ctx):
    import concourse.bass as bass
    import concourse.mybir as mybir

    dt = mybir.dt
    AF = mybir.ActivationFunctionType
    OP = mybir.AluOpType

    # ---- DRAM I/O ----
    d_x = nc.dram_tensor("xs", [NS, H], dt.float32, kind="ExternalInput").ap()
    d_wqkv = nc.dram_tensor("wqkv", [128, L, 3, 3, BH], dt.bfloat16, kind="ExternalInput").ap()
    d_wo = nc.dram_tensor("wo", [128, L, 3, BH], dt.bfloat16, kind="ExternalInput").ap()
    d_w1 = nc.dram_tensor("w1", [128, L, 3, 12, 128], dt.bfloat16, kind="ExternalInput").ap()
    d_w2 = nc.dram_tensor("w2", [128, L, 12, BH], dt.bfloat16, kind="ExternalInput").ap()
    d_wproj = nc.dram_tensor("wproj", [128, 8, P_SEED * BH], dt.bfloat16, kind="ExternalInput").ap()
    d_wlm = nc.dram_tensor("wlm", [128, 3, V], dt.bfloat16, kind="ExternalInput").ap()
    d_identf = nc.dram_tensor("identf", [128, 64], dt.float32, kind="ExternalInput").ap()
    d_identb = nc.dram_tensor("identb", [128, 64], dt.bfloat16, kind="ExternalInput").ap()
    d_logits = nc.dram_tensor("logits", [NS, S_C, V], dt.float32, kind="ExternalOutput").ap()

    # ---- pools ----
    singles = ctx.enter_context(tc.tile_pool(name="singles", bufs=1))
    wpool = ctx.enter_context(tc.tile_pool(name="work", bufs=2))
    xpool = ctx.enter_context(tc.tile_pool(name="xi", bufs=3))
    spool = ctx.enter_context(tc.tile_pool(name="small", bufs=3))
    bigpool = ctx.enter_context(tc.tile_pool(name="big", bufs=2))
    pmm = ctx.enter_context(tc.tile_pool(name="pmm", bufs=4, space="PSUM"))
    ptr = ctx.enter_context(tc.tile_pool(name="ptr", bufs=2, space="PSUM"))
    pg = ctx.enter_context(tc.tile_pool(name="pg", bufs=2, space="PSUM"))

    # ---- persistent SBUF ----
    w_qkv = singles.tile([128, L, 3, 3, BH], dt.bfloat16)
    w_o = singles.tile([128, L, 3, BH], dt.bfloat16)
    w_1 = singles.tile([128, L, 3, 12, 128], dt.bfloat16)
    w_2 = singles.tile([128, L, 12, BH], dt.bfloat16)
    w_lm = singles.tile([128, 3, V], dt.bfloat16)
    identf = singles.tile([128, 64], dt.float32)
    identb = singles.tile([128, 64], dt.bfloat16)
    # KV caches: layer pairs share a tile; even layer in rows 0:64, odd in 64:128
    kcache = [singles.tile([128, T, BH], dt.bfloat16, name=f"kc{i}", tag=f"kc{i}") for i in range(2)]
    vcache = [singles.tile([128, BH, T], dt.bfloat16, name=f"vc{i}", tag=f"vc{i}") for i in range(2)]  # t innermost

    for dst, src in ((w_qkv, d_wqkv), (w_o, d_wo), (w_1, d_w1), (w_2, d_w2),
                     (w_lm, d_wlm), (identf, d_identf), (identb, d_identb)):
        nc.sync.dma_start(out=dst, in_=src)

    # consts / state
    czero = singles.tile([128, 1], dt.float32)
    ceps = singles.tile([128, 1], dt.float32)
    cone = singles.tile([128, 1], dt.float32)
    cneg = singles.tile([128, 1], dt.float32)
    f_t = singles.tile([64, 1], dt.float32)   # finished (0/1)
    m_t = singles.tile([64, 1], dt.float32)   # 1 - finished
    nc.vector.memset(czero, 0.0)
    nc.vector.memset(ceps, 1e-5)
    nc.vector.memset(cone, 1.0)
    nc.vector.memset(cneg, -float(EOS))
    nc.vector.memset(f_t, 0.0)
    nc.vector.memset(m_t, 1.0)

    def transpose_384(dst_sb, src_ap, rows, fp32, nchunk=3):
        """src [64, nchunk*128] (at partition base rows.start) -> dst_sb [128, nchunk, 64]."""
        tr = ptr.tile([128, 3, 64], dt.float32 if fp32 else dt.bfloat16, tag="tr")
        ident = identf if fp32 else identb
        for c in range(nchunk):
            nc.tensor.transpose(tr[:, c, :], src_ap[:, c * 128:(c + 1) * 128],
                                ident[rows, :])
        nc.scalar.copy(dst_sb[:, 0:nchunk, :], tr[:, 0:nchunk, :])

    def rms_to_bf16(xi, h_out):
        """h_out [64, BH] bf16 = xi * rsqrt(mean(xi^2)+eps); gain folded in W."""
        sq = bigpool.tile([64, BH], dt.bfloat16, tag="prod", bufs=1)
        ssq = spool.tile([64, 1], dt.float32, tag="ssq")
        nc.scalar.activation(sq, xi, AF.Square, accum_out=ssq)
        sd = spool.tile([64, 1], dt.float32, tag="sd")
        nc.scalar.activation(sd, ssq, AF.Sqrt, bias=ceps[0:64, :], scale=1.0 / BH)
        r = spool.tile([64, 1], dt.float32, tag="r")
        nc.vector.reciprocal(r, sd)
        nc.vector.tensor_scalar_mul(h_out, xi, r)

    def block(l, it, xi):
        """One transformer block for position `it`, layer l. xi: [64, BH] f32 AP."""
        pos, nk = it, it + 1
        hi = (l % 2 == 1)
        rows = slice(64, 128) if hi else slice(0, 64)
        tp = (0, 64) if hi else None
        kc = kcache[l // 2]
        vc = vcache[l // 2]

        # --- attn RMS + hT ---
        h = wpool.tile([64, BH], dt.bfloat16, tag="h")
        rms_to_bf16(xi, h)
        hT = wpool.tile([128, 3, 64], dt.bfloat16, tag="hT")
        transpose_384(hT, h, slice(0, 64), False)

        # --- QKV ---
        qkv_ps = [pmm.tile([128, BH], dt.float32, name=f"qkv{j}", tag="mm") for j in range(3)]
        for c in range(3):
            for j in range(3):
                nc.tensor.matmul(qkv_ps[j][rows, :], lhsT=hT[:, c, :],
                                 rhs=w_qkv[:, l, c, j, :],
                                 start=(c == 0), stop=(c == 2), tile_position=tp)
        q_sb = wpool.tile([128, BH], dt.bfloat16, tag="q")
        nc.scalar.copy(q_sb[rows, :], qkv_ps[0][rows, :])
        nc.scalar.copy(kc[rows, pos, :], qkv_ps[1][rows, :])
        nc.scalar.copy(vc[rows, :, pos], qkv_ps[2][rows, :])

        # --- scores = q . k ---
        prod = bigpool.tile([128, T, BH], dt.bfloat16, tag="prod", bufs=1)
        q_bc = q_sb[rows, :].unsqueeze(1).broadcast_to([64, nk, BH])
        nc.vector.tensor_mul(prod[rows, :nk, :], kc[rows, :nk, :], q_bc)
        pv4 = prod[rows, :nk, :].rearrange("p t (h d) -> p t h d", d=HD)
        nc.vector.tensor_add(pv4[:, :, :, 0:24], pv4[:, :, :, 0:24], pv4[:, :, :, 24:48])
        nc.vector.tensor_add(pv4[:, :, :, 0:12], pv4[:, :, :, 0:12], pv4[:, :, :, 12:24])
        sc = wpool.tile([128, T, NH], dt.float32, tag="sc")
        nc.vector.reduce_sum(sc[rows, :nk, :], pv4[:, :, :, 0:12],
                             axis=mybir.AxisListType.X)
        # --- softmax (no max-sub; |scores*SCALE| ~ 1) ---
        e_t = wpool.tile([128, T, NH], dt.float32, tag="e")
        nc.scalar.activation(e_t[rows, :nk, :], sc[rows, :nk, :], AF.Exp,
                             bias=czero[rows, :], scale=SCALE)
        den = spool.tile([128, NH], dt.float32, tag="den")
        nc.vector.reduce_sum(den[rows, :], e_t[rows, :nk, :].transpose([0, 2, 1]),
                             axis=mybir.AxisListType.X)
        rden = spool.tile([128, NH], dt.float32, tag="rden")
        nc.vector.reciprocal(rden[rows, :], den[rows, :])
        p_sb = wpool.tile([128, NH, T], dt.bfloat16, tag="p")
        nc.vector.tensor_mul(
            p_sb[rows, :, :nk].transpose([0, 2, 1]),      # [64, nk, 8] view
            e_t[rows, :nk, :],
            rden[rows, :].unsqueeze(1).broadcast_to([64, nk, NH]))
        # --- o = p . v ---
        prod2 = bigpool.tile([128, NH, HD, T], dt.bfloat16, tag="prod", bufs=1)
        nc.vector.tensor_mul(
            prod2[rows, :, :, :nk],
            vc[rows, :, :nk].rearrange("p (h d) t -> p h d t", d=HD),
            p_sb[rows, :, :nk].unsqueeze(2).broadcast_to([64, NH, HD, nk]))
        o_f = wpool.tile([128, BH], dt.float32, tag="o", bufs=1)
        w = nk
        pv = prod2[rows, :, :, :]
        while w > 4:
            a = ((w + 1) // 2 + 1) // 2 * 2      # even split point >= ceil(w/2)
            rem = w - a
            nc.vector.tensor_add(pv[:, :, :, 0:rem], pv[:, :, :, 0:rem],
                                 pv[:, :, :, a:w])
            w = a
        nc.vector.reduce_sum(o_f[rows, :], pv[:, :, :, 0:w],
                             axis=mybir.AxisListType.X)

        # --- out proj + residual ---
        oT = wpool.tile([128, 3, 64], dt.bfloat16, tag="hT")
        transpose_384(oT, o_f[rows, :], rows, True)
        o_ps = pmm.tile([128, BH], dt.float32, tag="mm")
        for c in range(3):
            nc.tensor.matmul(o_ps[0:64, :], lhsT=oT[:, c, :], rhs=w_o[:, l, c, :],
                             start=(c == 0), stop=(c == 2))
        x1 = xpool.tile([64, BH], dt.float32, tag="xi")
        nc.vector.tensor_add(x1, xi, o_ps[0:64, :])

        # --- FFN ---
        h2 = wpool.tile([64, BH], dt.bfloat16, tag="h")
        rms_to_bf16(x1, h2)
        h2T = wpool.tile([128, 3, 64], dt.bfloat16, tag="hT")
        transpose_384(h2T, h2, slice(0, 64), False)
        g_sb = wpool.tile([128, 12, 64], dt.bfloat16, tag="g")
        for half in range(2):
            g_ps = pg.tile([128, 6, 64], dt.float32, tag="g")
            for mi in range(6):
                mt = half * 6 + mi
                for c in range(3):
                    nc.tensor.matmul(g_ps[:, mi, :], lhsT=w_1[:, l, c, mt, :],
                                     rhs=h2T[:, c, :],
                                     start=(c == 0), stop=(c == 2))
            nc.scalar.activation(g_sb[:, half * 6:(half + 1) * 6, :], g_ps,
                                 AF.Gelu, bias=czero, scale=1.0)
        f2_ps = pmm.tile([128, BH], dt.float32, tag="mm")
        for mt in range(12):
            nc.tensor.matmul(f2_ps[0:64, :], lhsT=g_sb[:, mt, :], rhs=w_2[:, l, mt, :],
                             start=(mt == 0), stop=(mt == 11))
        x2 = xpool.tile([64, BH], dt.float32, tag="xi")
        nc.vector.tensor_add(x2, x1, f2_ps[0:64, :])
        return x2

    # ---- projection of x into seed positions ----
    xs_sb = bigpool.tile([64, H], dt.float32, tag="xload", bufs=1)
    nc.sync.dma_start(out=xs_sb, in_=d_x)
    xsT = wpool.tile([128, 8, 64], dt.bfloat16, tag="xsT")
    for c0 in range(0, 8, 2):
        transpose_384(xsT[:, c0:c0 + 2, :], xs_sb[:, c0 * 128:(c0 + 2) * 128],
                      slice(0, 64), True, nchunk=2)
    pp = [pmm.tile([128, 512], dt.float32, name=f"pp{j}", tag="mm") for j in range(3)]
    for c in range(8):
        wpb = bigpool.tile([128, P_SEED * BH], dt.bfloat16, tag="wpb")
        nc.sync.dma_start(out=wpb, in_=d_wproj[:, c, :])
        for j in range(3):
            nc.tensor.matmul(pp[j][0:64, :], lhsT=xsT[:, c, :],
                             rhs=wpb[:, j * 512:(j + 1) * 512],
                             start=(c == 0), stop=(c == 7))
    x0_sb = bigpool.tile([64, P_SEED * BH], dt.float32, tag="x0", bufs=1)
    for j in range(3):
        nc.scalar.copy(x0_sb[:, j * 512:(j + 1) * 512], pp[j][0:64, :])

    # ---- decode loop ----
    x0v = x0_sb.rearrange("p (s d) -> p s d", d=BH)
    xi = x0v[:, 0, :]
    for it in range(NIT):
        for l in range(L):
            xi = block(l, it, xi)
        if it < P_SEED - 1:
            xi = x0v[:, it + 1, :]
            continue
        gen = xi
        step = it - (P_SEED - 1)
        # masked gen -> next input (+ logits source). m read BEFORE update.
        gm = xpool.tile([64, BH], dt.float32, tag="xi")
        nc.scalar.activation(gm, gen, AF.Copy, scale=m_t)
        # logits row
        gT = wpool.tile([128, 3, 64], dt.bfloat16, tag="hT")
        transpose_384(gT, gm, slice(0, 64), True)
        lm_ps = pmm.tile([128, V], dt.float32, tag="mm")
        for c in range(3):
            nc.tensor.matmul(lm_ps[0:64, :], lhsT=gT[:, c, :], rhs=w_lm[:, c, :],
                             start=(c == 0), stop=(c == 2))
        lm_sb = wpool.tile([64, V], dt.float32, tag="lm")
        nc.scalar.copy(lm_sb, lm_ps[0:64, :])
        nc.sync.dma_start(out=d_logits[:, step, :], in_=lm_sb)
        # finished update from argmax(gen)
        mx8 = spool.tile([64, 8], dt.float32, tag="mx8")
        nc.vector.max(mx8, gen)
        idx8 = spool.tile([64, 8], dt.uint32, tag="idx8")
        nc.vector.max_index(idx8, mx8, gen)
        idxf = spool.tile([64, 1], dt.float32, tag="idxf")
        nc.vector.tensor_copy(idxf, idx8[:, 0:1])
        t1 = spool.tile([64, 1], dt.float32, tag="t1")
        nc.scalar.activation(t1, idxf, AF.Square, bias=cneg[0:64, :], scale=1.0)
        eq = spool.tile([64, 1], dt.float32, tag="eq")
        nc.scalar.activation(eq, t1, AF.Relu, bias=cone[0:64, :], scale=-1.0)
        eq2 = spool.tile([64, 1], dt.float32, tag="eq2")
        nc.vector.tensor_mul(eq2, eq, m_t)
        nc.vector.tensor_add(f_t, f_t, eq2)
        nc.scalar.activation(m_t, f_t, AF.Identity, bias=cone[0:64, :], scale=-1.0)
        xi = gm


_CACHE = {}


def _get_compiled():
    if "nc" in _CACHE:
        return _CACHE["nc"]
    from contextlib import ExitStack

    import concourse.bacc as bacc
    import concourse.tile as tile

    nc = bacc.Bacc("TRN2", target_bir_lowering=False, debug=False,
                   num_devices=NCORES)
    with tile.TileContext(nc) as tc:
        with ExitStack() as ctx:
            _build(nc, tc, ctx)
    nc.compile()
    _CACHE["nc"] = nc
    return nc


def kernel(**inputs):
    from concourse.bass_utils import run_bass_kernel_spmd

    shared, xs = _pack_inputs(
        inputs["x"], inputs["Wproj"], inputs["attn_norm"], inputs["Wq"],
        inputs["Wk"], inputs["Wv"], inputs["Wo"], inputs["ffn_norm"],
        inputs["W1"], inputs["W2"], inputs["Wlm"])

    nc = _get_compiled()
    in_maps = [dict(shared, xs=xs[c]) for c in range(NCORES)]
    res = run_bass_kernel_spmd(nc, in_maps, core_ids=list(range(NCORES)),
                               trace=bool(int(os.environ.get("KERNEL_TRACE", "0"))))
    logits = np.concatenate([r["logits"] for r in res.results], axis=0)
    _CACHE["last_exec_ns"] = res.exec_time_ns
    return logits.reshape(B, S, S_C, V).astype(F32)


if __name__ == "__main__":
    nc = _get_compiled()
    print("built + compiled OK")

